# revision 7
# baseline (speedup 1.0000x reference)
"""NonLocalBlock Trainium2 kernel (8 NeuronCores, sequence-parallel).

Reference computation (B=8, L=2048, C=512, CI=256):
    x = in @ Wx + bx; y = in @ Wy + by; o = in @ Wo + bo
    scores[b,l,m] = x[b,l,:] . y[b,m,:]
    attn = softmax(scores, axis=0)          # over the BATCH axis
    out = in + attn @ o

Sharding: softmax couples only the batch axis, which is elementwise in
(l, m) — so shard l across the 8 cores (LS=256 rows each) and keep all
batches per core.  No collectives needed.

Algebra used per core (biases are zero in this problem; asserted):
    M = Wx @ Wy^T                      [C, C]
    z[b,l,:]  = in[b,l,:] @ M          (only for own l-shard)
    scoresT[b][m,l] = in[b,m,:] . z[b,l,:]
    softmax over b (elementwise in (m,l))
    tT[b][c,l] = sum_m in[b,m,c] * attn[b][m,l]
    out[b,l,:] = in[b,l,:] + tT[b].T @ Wo

Precision: scores chain in float32r (TF32-like, full PE rate at N>=256),
attn-side matmuls in bf16 (errors average over the 2048-term m-sum),
softmax and residual in f32.
"""
import sys

sys.path.insert(0, "/opt/trn_rl_repo")

import numpy as np
import ml_dtypes

import concourse.bass as bass  # noqa: F401  (engine classes)
import concourse.mybir as mybir
from concourse import bacc
from concourse.tile import TileContext
from concourse.bass_utils import run_bass_kernel_spmd

B, L, C, CI = 8, 2048, 512, 256
NCORES = 8
LS = L // NCORES          # 256 l rows per core
MCH = 256                 # m-chunk size for the softmax sweep
NCH = L // MCH            # 8 chunks
P = 128

f32 = mybir.dt.float32
f32r = mybir.dt.float32r
bf16 = mybir.dt.bfloat16
EXP = mybir.ActivationFunctionType.Exp


def _round_tf32(x: np.ndarray) -> np.ndarray:
    """Round fp32 to the FP32R (11-bit mantissa) grid, nearest-even."""
    b = np.ascontiguousarray(x, dtype=np.float32).view(np.uint32)
    lsb = (b >> 12) & 1
    r = (b + np.uint32(0x7FF) + lsb) & np.uint32(0xFFFFF000)
    return r.view(np.float32)


def _build_program():
    nc = bacc.Bacc("TRN2")

    xT_chunks = nc.dram_tensor("xT_chunks", [B, NCH, C, MCH], f32r, kind="ExternalInput")
    xTs = nc.dram_tensor("xTs", [B, C, LS], f32r, kind="ExternalInput")
    in_bf = nc.dram_tensor("in_bf", [B, L, C], bf16, kind="ExternalInput")
    resid = nc.dram_tensor("resid", [B, LS, C], f32, kind="ExternalInput")
    wxt = nc.dram_tensor("wxt", [CI, C], f32r, kind="ExternalInput")
    wyt = nc.dram_tensor("wyt", [CI, C], f32r, kind="ExternalInput")
    wo_bf = nc.dram_tensor("wo_bf", [C, C], bf16, kind="ExternalInput")
    out = nc.dram_tensor("out", [B, LS, C], f32, kind="ExternalOutput")

    with TileContext(nc) as tc:
        with tc.tile_pool(name="persist", bufs=1) as pp, \
             tc.tile_pool(name="xq", bufs=3) as xqp, \
             tc.tile_pool(name="ib", bufs=2) as ibp, \
             tc.tile_pool(name="sc", bufs=1) as scp, \
             tc.tile_pool(name="stat", bufs=2) as stp, \
             tc.tile_pool(name="rb", bufs=2) as rbp:

            # ---- persistent SBUF tiles --------------------------------
            # zT[b]: [c'(4 ktiles stacked), l(256)] cols b*1024 + kt*256
            zT = pp.tile([P, B * 4 * LS], f32r, tag="zT")
            # tT[b]: [c(4 ctiles stacked), l(256)] cols b*1024 + ct*256
            tT = pp.tile([P, B * 4 * LS], bf16, tag="tT")
            # attn[b]: [m within tile, mt(16 tiles stacked) x l(256)]
            attn = [pp.tile([P, (L // P) * LS], bf16, tag=f"attn{b}",
                            name=f"attn{b}") for b in range(B)]
            # weights
            wo_t = pp.tile([P, 4 * C], bf16, tag="wo")     # [c(4kt), d]
            setup_ctx = tc.tile_pool(name="setup", bufs=1)
            sup = setup_ctx.__enter__()
            wxt_t = sup.tile([P, 2 * C], f32r, tag="wxt")   # [i(2kt), c]
            wyt_t = sup.tile([P, 2 * C], f32r, tag="wyt")
            m_sb = sup.tile([P, 4 * C], f32r, tag="m_sb")   # [c(4 ct), c'(512)]

            nc.sync.dma_start(wxt_t[:].rearrange("p (t c) -> p t c", t=2),
                              wxt.rearrange("(t p) c -> p t c", p=P))
            nc.sync.dma_start(wyt_t[:].rearrange("p (t c) -> p t c", t=2),
                              wyt.rearrange("(t p) c -> p t c", p=P))
            nc.sync.dma_start(wo_t[:].rearrange("p (t c) -> p t c", t=4),
                              wo_bf.rearrange("(t p) c -> p t c", p=P))

            psA_ctx = tc.tile_pool(name="psA", bufs=2, space="PSUM")
            psb = psA_ctx.__enter__()
            # ---- M = Wx @ Wy^T  [c, c'] -------------------------------
            for ct in range(4):
                pm = psb.tile([P, 4 * LS], f32, tag="pbig")
                for it in range(2):
                    nc.tensor.matmul(
                        pm[:, 0:C],
                        wxt_t[:, it * C + ct * P:it * C + (ct + 1) * P],
                        wyt_t[:, it * C:(it + 1) * C],
                        start=(it == 0), stop=(it == 1))
                nc.scalar.copy(m_sb[:, ct * C:(ct + 1) * C], pm[:, 0:C])  # f32 -> f32r

            # ---- zT[b] = M^T-contraction with own l-shard -------------
            for b in range(B):
                xs = xqp.tile([P, 4 * LS], f32r, tag="xq")
                nc.sync.dma_start(xs[:].rearrange("p (t l) -> p t l", t=4),
                                  xTs[b].rearrange("(t p) l -> p t l", p=P))
                pz = psb.tile([P, 4 * LS], f32, tag="pbig")
                for cpt in range(4):        # c' output tile
                    for ct in range(4):     # c contraction tile
                        nc.tensor.matmul(
                            pz[:, cpt * LS:(cpt + 1) * LS],
                            m_sb[:, ct * C + cpt * P:ct * C + (cpt + 1) * P],
                            xs[:, ct * LS:(ct + 1) * LS],
                            start=(ct == 0), stop=(ct == 3))
                nc.scalar.copy(zT[:, b * 1024:(b + 1) * 1024], pz[:])  # -> f32r

            psA_ctx.__exit__(None, None, None)
            setup_ctx.__exit__(None, None, None)
            psC_ctx = tc.tile_pool(name="psC", bufs=4, space="PSUM")
            pss = psC_ctx.__enter__()
            # ---- scores + softmax over b, m-chunk sweep ---------------
            for q in range(NCH):
                sc = scp.tile([P, B * 2 * MCH], f32, tag="sc")      # [128, 8*512]
                mxs = stp.tile([P, 1024], f32, tag="mxs")           # mx | S
                for b in range(B):
                    xq = xqp.tile([P, 4 * MCH], f32r, tag="xq")     # [c'(4kt), m(256)]
                    nc.sync.dma_start(xq[:].rearrange("p (t m) -> p t m", t=4),
                                      xT_chunks[b, q].rearrange("(t p) m -> p t m", p=P))
                    for mtl in range(2):
                        ps_ = pss.tile([P, MCH], f32, tag="ps_s", name=f"pss{q}_{b}_{mtl}")
                        for kt in range(4):
                            nc.tensor.matmul(
                                ps_[:],
                                xq[:, kt * MCH + mtl * P:kt * MCH + (mtl + 1) * P],
                                zT[:, b * 1024 + kt * LS:b * 1024 + (kt + 1) * LS],
                                start=(kt == 0), stop=(kt == 3))
                        nc.scalar.copy(sc[:, b * 512 + mtl * LS:b * 512 + (mtl + 1) * LS], ps_[:])
                    if b == 0:
                        nc.vector.tensor_copy(mxs[:, 0:512], sc[:, 0:512])
                    else:
                        nc.vector.tensor_max(mxs[:, 0:512], mxs[:, 0:512],
                                             sc[:, b * 512:(b + 1) * 512])
                for b in range(B):
                    s_b = sc[:, b * 512:(b + 1) * 512]
                    nc.vector.tensor_sub(s_b, s_b, mxs[:, 0:512])
                    nc.scalar.activation(s_b, s_b, EXP)
                    if b == 0:
                        nc.vector.tensor_copy(mxs[:, 512:1024], sc[:, 0:512])
                    else:
                        nc.vector.tensor_add(mxs[:, 512:1024], mxs[:, 512:1024], s_b)
                nc.vector.reciprocal(mxs[:, 0:512], mxs[:, 512:1024])
                for b in range(B):
                    nc.vector.tensor_mul(attn[b][:, q * 512:(q + 1) * 512],
                                         sc[:, b * 512:(b + 1) * 512],
                                         mxs[:, 0:512])

            # ---- tT[b] = sum_m in[b,m,c] * attn[b][m,l]  (bf16) -------
            psC_ctx.__exit__(None, None, None)
            psT_ctx = tc.tile_pool(name="psT", bufs=1, space="PSUM")
            pst = psT_ctx.__enter__()
            psO_ctx = tc.tile_pool(name="psO", bufs=2, space="PSUM")
            pso = psO_ctx.__enter__()
            for b in range(B):
                pts = [pst.tile([P, LS], f32, tag=f"pt{ct}", name=f"pt{b}_{ct}")
                       for ct in range(4)]
                for half in range(2):
                    ibts = []
                    for j in range(2):
                        quarter = half * 2 + j
                        ibt = ibp.tile([P, 4 * C], bf16, tag=f"ib{j}",
                                       name=f"ib{b}_{quarter}")
                        nc.sync.dma_start(
                            ibt[:].rearrange("p (t c) -> p t c", t=4),
                            in_bf[b, quarter * 512:(quarter + 1) * 512, :]
                            .rearrange("(t p) c -> p t c", p=P))
                        ibts.append(ibt)
                    for ct in range(4):
                        for mtq in range(8):
                            mt = half * 8 + mtq
                            nc.tensor.matmul(
                                pts[ct][:],
                                ibts[mtq // 4][:, (mtq % 4) * C + ct * P:
                                               (mtq % 4) * C + (ct + 1) * P],
                                attn[b][:, mt * LS:(mt + 1) * LS],
                                start=(mt == 0), stop=(mt == 15))
                for ct in range(4):
                    nc.scalar.copy(tT[:, b * 1024 + ct * LS:
                                   b * 1024 + (ct + 1) * LS], pts[ct][:])  # -> bf16

            # ---- out[b] = resid + tT[b].T @ Wo ------------------------
            for b in range(B):
                rbt = rbp.tile([P, 2 * C], f32, tag="rb")  # [l(2lt), d(512)]
                nc.sync.dma_start(rbt[:].rearrange("p (t c) -> p t c", t=2),
                                  resid[b].rearrange("(t p) c -> p t c", p=P))
                for lt in range(2):
                    po = pso.tile([P, C], f32, tag="po")
                    for ct in range(4):
                        nc.tensor.matmul(
                            po[:],
                            tT[:, b * 1024 + ct * LS + lt * P:
                               b * 1024 + ct * LS + (lt + 1) * P],
                            wo_t[:, ct * C:(ct + 1) * C],
                            start=(ct == 0), stop=(ct == 3))
                    nc.vector.tensor_add(rbt[:, lt * C:(lt + 1) * C], po[:],
                                         rbt[:, lt * C:(lt + 1) * C])
                nc.sync.dma_start(out[b].rearrange("(t p) c -> p t c", p=P),
                                  rbt[:].rearrange("p (t c) -> p t c", t=2))
            psO_ctx.__exit__(None, None, None)
            psT_ctx.__exit__(None, None, None)

    nc.compile()
    return nc


_PROGRAM = None


def _get_program():
    global _PROGRAM
    if _PROGRAM is None:
        _PROGRAM = _build_program()
    return _PROGRAM


def _prepare_maps(inputs, Wx, bx, Wy, by, Wo, bo):
    if not (np.all(bx == 0) and np.all(by == 0) and np.all(bo == 0)):
        raise NotImplementedError("nonzero biases not supported by this kernel")
    inputs = np.ascontiguousarray(inputs, dtype=np.float32)
    inT = np.ascontiguousarray(inputs.transpose(0, 2, 1))        # [B, C, L]
    inT_r = _round_tf32(inT)
    xt_chunks = np.ascontiguousarray(
        inT_r.reshape(B, C, NCH, MCH).transpose(0, 2, 1, 3))     # [B, NCH, C, MCH]
    in_bf = inputs.astype(ml_dtypes.bfloat16)
    wxt = _round_tf32(np.ascontiguousarray(np.asarray(Wx, np.float32).T))
    wyt = _round_tf32(np.ascontiguousarray(np.asarray(Wy, np.float32).T))
    wo_b = np.asarray(Wo, np.float32).astype(ml_dtypes.bfloat16)

    in_maps = []
    for r in range(NCORES):
        sl = slice(r * LS, (r + 1) * LS)
        in_maps.append({
            "xT_chunks": xt_chunks,
            "xTs": np.ascontiguousarray(inT_r[:, :, sl]),
            "in_bf": in_bf,
            "resid": np.ascontiguousarray(inputs[:, sl, :]),
            "wxt": wxt,
            "wyt": wyt,
            "wo_bf": wo_b,
        })
    return in_maps


def kernel(inputs, Wx, bx, Wy, by, Wo, bo):
    nc = _get_program()
    in_maps = _prepare_maps(inputs, Wx, bx, Wy, by, Wo, bo)
    res = run_bass_kernel_spmd(nc, in_maps, list(range(NCORES)))
    return np.concatenate([res.results[r]["out"] for r in range(NCORES)], axis=1)
